# revision 28
# baseline (speedup 1.0000x reference)
"""ContextualNeuronPool Trainium2 kernel (8-core SPMD), v4.

Math (per token t, K=8 selected pool entries p_k = idx[t,k], w = softmax(pattern_weights[t])):
    combined[t, f] = sum_k w_k * bp_eff[p_k, f]                  (base term, via routing matrix A)
                   + (sum_k w_k * (G[p_k] @ x[t])) @ adj_proj    (modulation term, via MoE grouping)
    out[t] = gelu(combined[t]) @ W2^T + w2_b
where G[p] = cm_w block [64, 1024] and bp_eff folds the cm_b bias (host side).
Host folds softmax weights into the xgt pair columns and builds A^T.

v4: the AllGather of the full pair table (4.85MB) + 32 indirect gathers is
replaced by:
  - per-slot indirect SCATTER of computed pair vectors into a dest-core-major
    table (runs on gpsimd during phase A),
  - one small AllToAll (8 x 640 rows x 128B = 0.64MB),
  - one flat contiguous receive load, and
  - the k-sum as a 40-chunk GEMM against per-chunk onehot matrices (built on
    DVE from host-shipped token indices; permutation is free in the GEMM).
All operands stay in natural layouts; bf16 throughout.
"""

import numpy as np
import ml_dtypes

import concourse.bacc as bacc
import concourse.bass as bass
import concourse.tile as tile
import concourse.mybir as mybir
from concourse.bass_utils import run_bass_kernel_spmd
from concourse.masks import make_identity

BF16 = mybir.dt.bfloat16
F32 = mybir.dt.float32
I32 = mybir.dt.int32
AF = mybir.ActivationFunctionType
ALU = mybir.AluOpType

POOL, D, DFF, M = 512, 1024, 4096, 64
B, S, K = 2, 2048, 8
NCORES = 8
NTOK = B * S                  # 4096 tokens
T = NTOK // NCORES            # 512 tokens per core
EPC = POOL // NCORES          # 64 experts (pool entries) per core
DC = D // 128                 # 8 contraction chunks
TT = T // 128                 # 4 token tiles per core
PC = POOL // 128              # 4 pool chunks
FT = DFF // 128               # 32 d_ff tiles
GRP = 16                      # expert slots per group
NG = EPC // GRP               # 4 groups
HGRP = GRP // 2               # half-group (load granularity)
PADR = 640                    # padded rows per (src, dst) AllToAll block
RTOT = NCORES * PADR          # 5120 total table rows
RJ = RTOT // 128              # 40 recv chunks (rows per partition)


def _build_program(slot_info):
    slot_sizes, slot_counts = slot_info
    slot_off = np.concatenate([[0], np.cumsum(slot_sizes)]).astype(int)
    TW = int(slot_off[-1])
    ho = [int(slot_off[h * HGRP]) for h in range(2 * NG + 1)]

    nc = bacc.Bacc("TRN2", target_bir_lowering=False, debug=False, num_devices=NCORES)

    xgt_d = nc.dram_tensor("xgt", [128, DC * TW], BF16, kind="ExternalInput")
    cmt_d = nc.dram_tensor("cmt", [128, EPC * DC * M], BF16, kind="ExternalInput")
    bp_d = nc.dram_tensor("bp", [128, PC * DFF], BF16, kind="ExternalInput")
    atT_d = nc.dram_tensor("atT", [128, PC * T], BF16, kind="ExternalInput")
    adj_d = nc.dram_tensor("adjp", [M, DFF], BF16, kind="ExternalInput")
    w2t_d = nc.dram_tensor("w2t", [128, FT * D], BF16, kind="ExternalInput")
    sidx_d = nc.dram_tensor("sidx", [128, EPC], I32, kind="ExternalInput")
    tokx_d = nc.dram_tensor("tokx", [128, RJ], F32, kind="ExternalInput")
    a2ain_d = nc.dram_tensor("a2ain", [RTOT + 128, M], BF16, kind="ExternalInput")  # zeros
    out_d = nc.dram_tensor("out", [T, D], BF16, kind="ExternalOutput")

    with tile.TileContext(nc) as tc:
        with tc.tile_pool(name="const", bufs=1) as const, \
             tc.tile_pool(name="pra", bufs=6) as pr_pool, \
             tc.tile_pool(name="outp", bufs=2) as out_pool, \
             tc.tile_pool(name="dram", bufs=1, space="DRAM") as dram:

            # ---------------- constants / small inputs ----------------
            ident = const.tile([128, 128], BF16)
            make_identity(nc, ident[:])
            iota_f = const.tile([128, T], F32, tag="iota")
            nc.gpsimd.iota(iota_f[:], pattern=[[1, T]], base=0, channel_multiplier=0,
                           allow_small_or_imprecise_dtypes=True)
            sidx_sb = const.tile([128, EPC], I32, tag="sidx")
            nc.scalar.dma_start(out=sidx_sb[:], in_=sidx_d[:, :])
            tokx_sb = const.tile([128, RJ], F32, tag="tokx")
            nc.scalar.dma_start(out=tokx_sb[:], in_=tokx_d[:, :])
            adj_sb = const.tile([M, DFF], BF16, tag="adj")
            nc.scalar.dma_start(out=adj_sb[:], in_=adj_d[:, :])

            a2a_in = dram.tile([RTOT + 128, M], BF16)
            a2a_out = dram.tile([RTOT, M], BF16)
            # zero the scatter table (DRAM->DRAM from the host zeros input):
            # unwritten rows must be 0.0, not uninitialized bits (NaN x 0 = NaN)
            nc.sync.dma_start(out=a2a_in[:], in_=a2ain_d[:, :])

            stage_tiles = []   # combined^T tiles [128 f, T]
            for ft in range(FT):
                stage_tiles.append(const.tile([128, T], BF16, tag=f"stg{ft}",
                                              name=f"stg{ft}"))

            with tc.tile_pool(name="bpat", bufs=1) as bpat, \
                 tc.tile_pool(name="xg", bufs=2) as xg_pool, \
                 tc.tile_pool(name="cm", bufs=2) as cm_pool:
                bp_all = bpat.tile([128, PC * DFF], BF16, tag="bpall")
                atT_all = bpat.tile([128, PC * T], BF16, tag="atall")

                # ---------------- phase A: per-slot modulation pair vectors ----------------
                with tc.tile_pool(name="psA", bufs=3, space="PSUM") as psA, \
                     tc.tile_pool(name="psB", bufs=4, space="PSUM") as psB:
                    for g in range(NG):
                        xgh, cmh = [], []
                        for h in range(2):
                            hi = 2 * g + h
                            hw = ho[hi + 1] - ho[hi]
                            xgt_ = xg_pool.tile([128, DC * hw], BF16, tag=f"xga{h}",
                                                name=f"xg{g}_{h}")
                            cmt_ = cm_pool.tile([128, HGRP * DC * M], BF16,
                                                tag=f"cma{h}", name=f"cm{g}_{h}")
                            nc.sync.dma_start(
                                out=xgt_[:], in_=xgt_d[:, DC * ho[hi]:DC * ho[hi + 1]])
                            nc.scalar.dma_start(
                                out=cmt_[:],
                                in_=cmt_d[:, hi * HGRP * DC * M:(hi + 1) * HGRP * DC * M])
                            xgh.append(xgt_)
                            cmh.append(cmt_)
                        if g == 1:
                            # deferred so the first xg/cm halves own the HBM
                            # bandwidth; still lands well before pass1
                            nc.scalar.dma_start(out=bp_all[:], in_=bp_d[:, :])
                            nc.scalar.dma_start(out=atT_all[:], in_=atT_d[:, :])
                        for s in range(GRP):
                            sl = g * GRP + s
                            m_s = int(slot_sizes[sl])
                            h = s // HGRP
                            hw = ho[2 * g + h + 1] - ho[2 * g + h]
                            lo = int(slot_off[sl] - ho[2 * g + h])
                            sidx = s % HGRP
                            ps = psA.tile([128, M], F32)
                            for j in range(DC):
                                nc.tensor.matmul(
                                    ps[:m_s, :],
                                    lhsT=xgh[h][:, j * hw + lo:j * hw + lo + m_s],
                                    rhs=cmh[h][:, (sidx * DC + j) * M:(sidx * DC + j + 1) * M],
                                    start=(j == 0), stop=(j == DC - 1))
                            pr = pr_pool.tile([128, M], BF16, tag="pr")
                            nc.vector.tensor_copy(out=pr[:m_s, :], in_=ps[:m_s, :])
                            # scatter real rows into the dest-major A2A table
                            nc.gpsimd.indirect_dma_start(
                                out=a2a_in[:], out_offset=bass.IndirectOffsetOnAxis(
                                    ap=sidx_sb[:m_s, sl:sl + 1], axis=0),
                                in_=pr[:m_s, :], in_offset=None)
                    # one small AllToAll delivers each core its tokens' pairs
                    nc.gpsimd.collective_compute(
                        "AllToAll", ALU.bypass,
                        replica_groups=[list(range(NCORES))],
                        ins=[a2a_in[0:RTOT, :].opt()],
                        outs=[a2a_out[:].opt()],
                    )

                    # ---- pass 1: base term combined^T = (A @ bp_eff)^T ----
                    for ft in range(FT):
                        psb = psB.tile([128, T], F32)
                        for pj in range(PC):
                            nc.tensor.matmul(
                                psb[:],
                                lhsT=bp_all[:, pj * DFF + ft * 128:pj * DFF + (ft + 1) * 128],
                                rhs=atT_all[:, pj * T:(pj + 1) * T],
                                start=(pj == 0), stop=(pj == PC - 1))
                        nc.scalar.activation(out=stage_tiles[ft][:], in_=psb[:],
                                             func=AF.Copy)

            with tc.tile_pool(name="oh", bufs=1) as oh_pool, \
                 tc.tile_pool(name="w2s", bufs=1) as w2_pool, \
                 tc.tile_pool(name="rcv", bufs=1) as rcv_pool:
                # onehot chunks for the k-sum GEMM (DVE, overlaps pass1)
                o2 = []
                for j in range(RJ):
                    t_ = oh_pool.tile([128, T], BF16, tag=f"oh{j}", name=f"oh{j}")
                    nc.vector.tensor_scalar(out=t_[:], in0=iota_f[:],
                                            scalar1=tokx_sb[:, j:j + 1], scalar2=None,
                                            op0=ALU.is_equal)
                    o2.append(t_)
                # W2 weights: all 32 chunks resident (arena reuses freed pools)
                w2c = []
                for fc in range(FT):
                    t_ = w2_pool.tile([128, D], BF16, tag=f"w2c{fc}", name=f"w2c{fc}")
                    nc.sync.dma_start(out=t_[:], in_=w2t_d[:, fc * D:(fc + 1) * D])
                    w2c.append(t_)
                # receive: one flat load (partition p <- rows p*40 .. p*40+40)
                recv = rcv_pool.tile([128, RJ * M], BF16, tag="recv")
                nc.scalar.dma_start(out=recv[:], in_=a2a_out[:, :])

                # ---------------- back half ----------------
                wq = const.tile([M, T], BF16, tag="wqT")
                with tc.tile_pool(name="psW", bufs=1, space="PSUM") as psW, \
                     tc.tile_pool(name="psC", bufs=3, space="PSUM") as psC:
                    # k-sum GEMM: wq[m, t] = sum_j recv_j^T @ onehot_j
                    psw = psW.tile([M, T], F32, tag="psw")
                    for j in range(RJ):
                        nc.tensor.matmul(psw[:], lhsT=recv[:, j * M:(j + 1) * M],
                                         rhs=o2[j][:], start=(j == 0), stop=(j == RJ - 1))
                    nc.vector.tensor_copy(out=wq[:], in_=psw[:])
                    # pass2: psc = adj_chunk @ wq + I @ stage ; gelu PSUM -> stage
                    for ft in range(FT):
                        psc = psC.tile([128, T], F32, tag="psc")
                        nc.tensor.matmul(psc[:], lhsT=adj_sb[:, ft * 128:(ft + 1) * 128],
                                         rhs=wq[:], start=True, stop=False)
                        nc.tensor.matmul(psc[:], lhsT=ident[:],
                                         rhs=stage_tiles[ft][:],
                                         start=False, stop=True)
                        nc.scalar.activation(out=stage_tiles[ft][:], in_=psc[:],
                                             func=AF.Gelu)

                # W2 (fc-outer): psO[(q,dd)] accumulate across all fc
                with tc.tile_pool(name="psO", bufs=1, space="PSUM") as psO_pool:
                    psO = {}
                    for q in range(TT):
                        for dd in range(2):
                            psO[(q, dd)] = psO_pool.tile(
                                [128, 512], F32, tag=f"o{q}_{dd}", name=f"ops{q}_{dd}")
                    for fc in range(FT):
                        for q in range(TT):
                            for dd in range(2):
                                nc.tensor.matmul(
                                    psO[(q, dd)][:],
                                    lhsT=stage_tiles[fc][:, q * 128:(q + 1) * 128],
                                    rhs=w2c[fc][:, dd * 512:(dd + 1) * 512],
                                    start=(fc == 0), stop=(fc == FT - 1))
                    for q in range(TT):
                        ob = out_pool.tile([128, D], BF16, tag="ob")
                        for dd in range(2):
                            nc.vector.tensor_copy(out=ob[:, dd * 512:(dd + 1) * 512],
                                                  in_=psO[(q, dd)][:])
                        nc.scalar.dma_start(out=out_d[q * 128:(q + 1) * 128, :], in_=ob[:])

    nc.compile()
    return nc


def _routing(idx):
    """Group (t, k) pairs by pool entry; per-core slot packing + A2A routing."""
    flat_e = idx.ravel()
    order = np.argsort(flat_e, kind="stable")  # pairs sorted by (expert, t, k)
    counts = np.bincount(flat_e, minlength=POOL)
    starts = np.zeros(POOL, dtype=np.int64)
    starts[1:] = np.cumsum(counts)[:-1]
    tok_sorted = (np.arange(NTOK * K, dtype=np.int64) // K)[order]

    slot_expert = np.zeros((NCORES, EPC), dtype=np.int64)
    for c in range(NCORES):
        cnt = counts[c * EPC:(c + 1) * EPC]
        slot_expert[c] = c * EPC + np.argsort(-cnt, kind="stable")
    slot_counts_pc = counts[slot_expert]                    # [NCORES, EPC]
    slot_sizes = ((slot_counts_pc.max(axis=0) + 15) // 16 * 16).astype(np.int64)
    slot_sizes = np.maximum(slot_sizes, 16)
    assert slot_sizes.max() <= 128, f"slot overflow {slot_sizes.max()}"
    slot_off = np.concatenate([[0], np.cumsum(slot_sizes)])
    TW = int(slot_off[-1])
    return order, counts, starts, tok_sorted, slot_expert, slot_counts_pc, slot_sizes, slot_off, TW


def _prepare_inputs(x, selected_indices, pattern_weights, base_patterns, cm_w, cm_b,
                    adj_proj, w2_w):
    bf = ml_dtypes.bfloat16
    x2 = np.ascontiguousarray(x.reshape(NTOK, D), dtype=np.float32)
    idx = np.ascontiguousarray(selected_indices.reshape(NTOK, K)).astype(np.int32)
    pw = np.ascontiguousarray(pattern_weights.reshape(NTOK, K), dtype=np.float32)

    pw_m = pw - pw.max(axis=1, keepdims=True)
    e = np.exp(pw_m)
    w = (e / e.sum(axis=1, keepdims=True)).astype(np.float32)      # [NTOK, K]

    bp_eff = base_patterns.astype(np.float32) + cm_b.reshape(POOL, M).astype(np.float32) @ adj_proj.astype(np.float32)
    bp_bf = bp_eff.astype(bf)
    adj_bf = adj_proj.astype(bf)
    x2t = x2.T                                                     # [D, NTOK] f32

    (order, counts, starts, tok_sorted, slot_expert, slot_counts_pc, slot_sizes,
     slot_off, TW) = _routing(idx)
    ho = slot_off[::HGRP]
    w_sorted = w.ravel()[order]

    bp_t = np.ascontiguousarray(
        bp_bf.reshape(PC, 128, DFF).transpose(1, 0, 2).reshape(128, PC * DFF))
    w2t = np.ascontiguousarray(
        w2_w.T.astype(bf).reshape(FT, 128, D).transpose(1, 0, 2).reshape(128, FT * D))

    cm3 = cm_w.reshape(POOL, M, D)
    zeros_a2a = np.zeros((RTOT + 128, M), dtype=bf)
    in_maps = []
    # per-core routing tables for the A2A
    #   sidx[c]: scatter row for each (slot, rank)
    #   tokx[d]: local token (or dummy) for each recv row, in [128, RJ] layout
    tokx_all = np.full((NCORES, RTOT), T + 88, dtype=np.float32)   # dummy
    # default every (rank, slot) to a trash row >= RTOT (outside the A2A input
    # range); real pairs overwrite below. Padded ranks thus scatter harmlessly.
    sidx_all = np.tile((RTOT + np.arange(128, dtype=np.int32))[None, :, None],
                       (NCORES, 1, EPC))
    for c in range(NCORES):
        rcount = np.zeros(NCORES, dtype=np.int64)
        for sl in range(EPC):
            e_ = int(slot_expert[c, sl])
            cnt = int(counts[e_])
            toks = tok_sorted[starts[e_]:starts[e_] + cnt]         # global tokens
            d = toks // T                                          # dest cores
            for i in range(cnt):
                di = int(d[i])
                r = rcount[di]; rcount[di] += 1
                row = di * PADR + r
                sidx_all[c, i, sl] = row
                # receiver d sees this pair at recv row c*PADR + r
                tokx_all[di, c * PADR + r] = float(toks[i] - di * T)
        assert rcount.max() <= PADR, f"A2A block overflow {rcount.max()}"

    for c in range(NCORES):
        xgt = np.zeros((128, DC * TW), dtype=bf)
        cmt = np.empty((128, EPC * DC * M), dtype=bf)
        for hi in range(2 * NG):
            hw = int(ho[hi + 1] - ho[hi])
            blk = np.zeros((D, hw), dtype=np.float32)
            for si in range(HGRP):
                sl = hi * HGRP + si
                e_ = int(slot_expert[c, sl])
                seg = slice(starts[e_], starts[e_] + counts[e_])
                toks = tok_sorted[seg]
                lo = int(slot_off[sl] - ho[hi])
                blk[:, lo:lo + len(toks)] = x2t[:, toks] * w_sorted[seg][None, :]
            xgt[:, DC * ho[hi]:DC * ho[hi + 1]] = (
                blk.reshape(DC, 128, hw).transpose(1, 0, 2).reshape(128, DC * hw)
            ).astype(bf)
        for sl in range(EPC):
            e_ = int(slot_expert[c, sl])
            cmt[:, sl * DC * M:(sl + 1) * DC * M] = (
                cm3[e_].T.reshape(DC, 128, M).transpose(1, 0, 2).reshape(128, DC * M)
            ).astype(bf)

        at = np.zeros((POOL, T), dtype=np.float32)
        tl = np.arange(c * T, (c + 1) * T)
        for k in range(K):
            np.add.at(at, (idx[tl, k], np.arange(T)), w[tl, k])
        atT = np.ascontiguousarray(
            at.astype(bf).reshape(PC, 128, T).transpose(1, 0, 2).reshape(128, PC * T))

        # tokx in [128, RJ]: row (p*RJ + j) -> [p, j]
        tokx = np.ascontiguousarray(tokx_all[c].reshape(128, RJ))
        in_maps.append({
            "xgt": xgt,
            "cmt": np.ascontiguousarray(cmt),
            "bp": bp_t,
            "atT": atT,
            "adjp": adj_bf,
            "w2t": w2t,
            "sidx": np.ascontiguousarray(sidx_all[c]),
            "tokx": tokx,
            "a2ain": zeros_a2a,
        })
    return in_maps, (slot_sizes, None)


def _run(inputs, trace=False):
    in_maps, slot_info = _prepare_inputs(
        inputs["x"], inputs["selected_indices"], inputs["pattern_weights"],
        inputs["base_patterns"], inputs["cm_w"], inputs["cm_b"],
        inputs["adj_proj"], inputs["w2_w"])
    nc = _build_program(slot_info)
    res = run_bass_kernel_spmd(nc, in_maps, core_ids=list(range(NCORES)), trace=trace)
    out = np.concatenate([res.results[c]["out"].astype(np.float32)
                          for c in range(NCORES)], axis=0)
    out = out + np.asarray(inputs["w2_b"], dtype=np.float32)[None, :]
    return out.reshape(B, S, D).astype(np.float32), res


def kernel(**inputs) -> np.ndarray:
    out, _ = _run(inputs, trace=False)
    return out


# revision 31
# speedup vs baseline: 1.6584x; 1.6584x over previous
"""ContextualNeuronPool Trainium2 kernel (8-core SPMD), v5.

Math (per token t, K=8 selected pool entries p_k = idx[t,k], w = softmax(pattern_weights[t])):
    combined[t, f] = sum_k w_k * bp_eff[p_k, f]                  (base term, via routing matrix A)
                   + (sum_k w_k * (G[p_k] @ x[t])) @ adj_proj    (modulation term, via MoE grouping)
    out[t] = gelu(combined[t]) @ W2^T + w2_b
where G[p] = cm_w block [64, 1024] and bp_eff folds the cm_b bias (host side).
Host folds softmax weights into the xgt pair columns and builds A^T.

Cross-core exchange of the per-pair modulation vectors (expert-sharded phase A
-> token-sharded back half):
  - phase A computes pair vectors per expert slot; each slot's rows are
    indirect-SCATTERed (gpsimd) into a dest-core-major table, with disjoint
    fake dep ranges so scatters pipeline without false WAW chains;
  - slots are split in NR=2 rounds; each round's table is AllGathered (fast
    Shared-output path) as soon as its slots finish -> overlaps phase A;
  - each core then pulls its own dest slice with 8 small indirect gathers per
    round (per-core offsets shipped as an input; 3 contiguous rows/partition);
  - the k-sum is a 48-chunk GEMM against onehot matrices built on DVE from
    host token indices (permutation free in the GEMM).
"""

import numpy as np
import ml_dtypes

import concourse.bacc as bacc
import concourse.bass as bass
import concourse.tile as tile
import concourse.mybir as mybir
from concourse.bass_utils import run_bass_kernel_spmd
from concourse.masks import make_identity

BF16 = mybir.dt.bfloat16
F32 = mybir.dt.float32
I32 = mybir.dt.int32
FP8 = mybir.dt.float8e4
AF = mybir.ActivationFunctionType
ALU = mybir.AluOpType

POOL, D, DFF, M = 512, 1024, 4096, 64
B, S, K = 2, 2048, 8
NCORES = 8
NTOK = B * S                  # 4096 tokens
T = NTOK // NCORES            # 512 tokens per core
EPC = POOL // NCORES          # 64 experts (pool entries) per core
DC = D // 128                 # 8 contraction chunks
TT = T // 128                 # 4 token tiles per core
PC = POOL // 128              # 4 pool chunks
FT = DFF // 128               # 32 d_ff tiles
GRP = 16                      # expert slots per group
NG = EPC // GRP               # 4 groups
HGRP = GRP // 2               # half-group (load granularity)
NR = 2                        # scatter/AllGather rounds
SPR = EPC // NR               # 32 slots per round
PADR = 384                    # rows per (src, dst) block per round
RTOT = NCORES * PADR          # 3072 rows per round table
RPS = PADR // 128             # 3 rows per partition per (round, src) gather
RJC = NCORES * RPS            # 24 ksum chunks per round
RJ = NR * RJC                 # 48 total ksum chunks


def _build_program(slot_info):
    slot_sizes, _ = slot_info
    slot_off = np.concatenate([[0], np.cumsum(slot_sizes)]).astype(int)
    TW = int(slot_off[-1])
    ho = [int(slot_off[h * HGRP]) for h in range(2 * NG + 1)]

    nc = bacc.Bacc("TRN2", target_bir_lowering=False, debug=False, num_devices=NCORES)

    xgt_d = nc.dram_tensor("xgt", [128, DC * TW], BF16, kind="ExternalInput")
    cmt_d = nc.dram_tensor("cmt", [128, EPC * DC * M], BF16, kind="ExternalInput")
    bp_d = nc.dram_tensor("bp", [128, PC * DFF], BF16, kind="ExternalInput")
    atT_d = nc.dram_tensor("atT", [128, PC * T], BF16, kind="ExternalInput")
    adj_d = nc.dram_tensor("adjp", [M, DFF], BF16, kind="ExternalInput")
    w2t_d = nc.dram_tensor("w2t", [128, FT * D], BF16, kind="ExternalInput")
    sidx_d = nc.dram_tensor("sidx", [128, EPC], I32, kind="ExternalInput")
    gof_d = nc.dram_tensor("gof", [128, NR * NCORES], I32, kind="ExternalInput")
    tokx_d = nc.dram_tensor("tokx", [128, RJ], F32, kind="ExternalInput")
    a2ain_d = nc.dram_tensor("a2ain", [RTOT + 128, M], BF16, kind="ExternalInput")
    out_d = nc.dram_tensor("out", [T, D], BF16, kind="ExternalOutput")

    with tile.TileContext(nc) as tc:
        with tc.tile_pool(name="const", bufs=1) as const, \
             tc.tile_pool(name="oh", bufs=1) as oh_pool, \
             tc.tile_pool(name="pra", bufs=24) as pr_pool, \
             tc.tile_pool(name="rcv", bufs=1) as rcv_pool, \
             tc.tile_pool(name="outp", bufs=2) as out_pool, \
             tc.tile_pool(name="dram", bufs=1, space="DRAM") as dram:

            # ---------------- constants / small inputs ----------------
            ident = const.tile([128, 128], BF16)
            make_identity(nc, ident[:])
            iota_f = const.tile([128, T], F32, tag="iota")
            nc.gpsimd.iota(iota_f[:], pattern=[[1, T]], base=0, channel_multiplier=0,
                           allow_small_or_imprecise_dtypes=True)
            sidx_sb = const.tile([128, EPC], I32, tag="sidx")
            nc.scalar.dma_start(out=sidx_sb[:], in_=sidx_d[:, :])
            gof_sb = const.tile([128, NR * NCORES], I32, tag="gof")
            nc.scalar.dma_start(out=gof_sb[:], in_=gof_d[:, :])
            tokx_sb = const.tile([128, RJ], F32, tag="tokx")
            nc.scalar.dma_start(out=tokx_sb[:], in_=tokx_d[:, :])
            adj_sb = const.tile([M, DFF], BF16, tag="adj")
            nc.scalar.dma_start(out=adj_sb[:], in_=adj_d[:, :])

            rtab, gtab = [], []
            for r in range(NR):
                t_ = dram.tile([RTOT + 128, M], BF16, tag=f"a2ai{r}", name=f"a2ai{r}")
                # zero: unwritten rows must be 0.0, not garbage (NaN x 0 = NaN)
                nc.sync.dma_start(out=t_[:], in_=a2ain_d[:, :])
                rtab.append(t_)
                gtab.append(dram.tile([NCORES * RTOT, M], BF16, addr_space="Shared",
                                      tag=f"ag{r}", name=f"ag{r}"))

            o2 = [None] * RJ

            def build_o2(j):
                t_ = oh_pool.tile([128, T], FP8, tag=f"oh{j}", name=f"oh{j}")
                nc.vector.tensor_scalar(out=t_[:], in0=iota_f[:],
                                        scalar1=tokx_sb[:, j:j + 1], scalar2=None,
                                        op0=ALU.is_equal)
                o2[j] = t_

            stage_tiles = []   # combined^T tiles [128 f, T]
            for ft in range(FT):
                stage_tiles.append(const.tile([128, T], BF16, tag=f"stg{ft}",
                                              name=f"stg{ft}"))

            with tc.tile_pool(name="bpat", bufs=1) as bpat, \
                 tc.tile_pool(name="xg", bufs=2) as xg_pool, \
                 tc.tile_pool(name="cm", bufs=2) as cm_pool:
                bp_all = bpat.tile([128, PC * DFF], BF16, tag="bpall")
                atT_all = bpat.tile([128, PC * T], BF16, tag="atall")

                # ---------------- phase A: per-slot modulation pair vectors ----------------
                with tc.tile_pool(name="psA", bufs=4, space="PSUM") as psA, \
                     tc.tile_pool(name="psB", bufs=4, space="PSUM") as psB:
                    for g in range(NG):
                        xgh, cmh = [], []
                        for h in range(2):
                            hi = 2 * g + h
                            hw = ho[hi + 1] - ho[hi]
                            xgt_ = xg_pool.tile([128, DC * hw], BF16, tag=f"xga{h}",
                                                name=f"xg{g}_{h}")
                            cmt_ = cm_pool.tile([128, HGRP * DC * M], BF16,
                                                tag=f"cma{h}", name=f"cm{g}_{h}")
                            nc.sync.dma_start(
                                out=xgt_[:], in_=xgt_d[:, DC * ho[hi]:DC * ho[hi + 1]])
                            nc.scalar.dma_start(
                                out=cmt_[:],
                                in_=cmt_d[:, hi * HGRP * DC * M:(hi + 1) * HGRP * DC * M])
                            xgh.append(xgt_)
                            cmh.append(cmt_)
                        if g == 1:
                            # deferred so the first xg/cm halves own the HBM bw
                            nc.scalar.dma_start(out=bp_all[:], in_=bp_d[:, :])
                            nc.scalar.dma_start(out=atT_all[:], in_=atT_d[:, :])
                        if g == NG // NR:
                            # round-0 slots all scattered: AllGather round 0
                            nc.gpsimd.collective_compute(
                                "AllGather", ALU.bypass,
                                replica_groups=[list(range(NCORES))],
                                ins=[rtab[0][0:RTOT, :].opt()],
                                outs=[gtab[0][:].opt()])
                        for s in range(GRP):
                            sl = g * GRP + s
                            m_s = int(slot_sizes[sl])
                            h = s // HGRP
                            hw = ho[2 * g + h + 1] - ho[2 * g + h]
                            lo = int(slot_off[sl] - ho[2 * g + h])
                            sidx = s % HGRP
                            ps = psA.tile([128, M], F32)
                            for j in range(DC):
                                nc.tensor.matmul(
                                    ps[:m_s, :],
                                    lhsT=xgh[h][:, j * hw + lo:j * hw + lo + m_s],
                                    rhs=cmh[h][:, (sidx * DC + j) * M:(sidx * DC + j + 1) * M],
                                    start=(j == 0), stop=(j == DC - 1))
                            pr = pr_pool.tile([128, M], BF16, tag="pr")
                            nc.vector.tensor_copy(out=pr[:m_s, :], in_=ps[:m_s, :])
                            # scatter into this round's table; disjoint fake dep
                            # ranges break the false WAW chain between scatters
                            fake_out = bass.AP(
                                tensor=rtab[sl // SPR][:].tensor, offset=0,
                                ap=[[M, 1], [1, M]],
                                dep_tracking_offset=(sl % SPR) * M)
                            nc.gpsimd.indirect_dma_start(
                                out=fake_out, out_offset=bass.IndirectOffsetOnAxis(
                                    ap=sidx_sb[:m_s, sl:sl + 1], axis=0),
                                in_=pr[:m_s, :], in_offset=None)
                            jlo = sl * RJ // EPC
                            jhi = (sl + 1) * RJ // EPC
                            for j in range(jlo, jhi):
                                build_o2(j)
                    nc.gpsimd.collective_compute(
                        "AllGather", ALU.bypass,
                        replica_groups=[list(range(NCORES))],
                        ins=[rtab[1][0:RTOT, :].opt()],
                        outs=[gtab[1][:].opt()])

                    # receive: 8 indirect gathers per round, each pulling this
                    # core's dest slice from one src block (3 rows/partition,
                    # per-core offsets from gof)
                    recv = []
                    for r in range(NR):
                        for srcc in range(NCORES):
                            t_ = rcv_pool.tile([128, RPS * M], BF16,
                                               tag=f"rc{r}_{srcc}", name=f"rc{r}_{srcc}")
                            nc.gpsimd.indirect_dma_start(
                                out=t_[:], out_offset=None,
                                in_=gtab[r][:],
                                in_offset=bass.IndirectOffsetOnAxis(
                                    ap=gof_sb[:, r * NCORES + srcc:r * NCORES + srcc + 1],
                                    axis=0))
                            recv.append(t_)

                    # ---- pass 1: base term combined^T = (A @ bp_eff)^T ----
                    for ft in range(FT):
                        psb = psB.tile([128, T], F32)
                        for pj in range(PC):
                            nc.tensor.matmul(
                                psb[:],
                                lhsT=bp_all[:, pj * DFF + ft * 128:pj * DFF + (ft + 1) * 128],
                                rhs=atT_all[:, pj * T:(pj + 1) * T],
                                start=(pj == 0), stop=(pj == PC - 1))
                        nc.vector.tensor_copy(out=stage_tiles[ft][:], in_=psb[:])

            with tc.tile_pool(name="w2s", bufs=1) as w2_pool:
                # W2 weights: all 32 chunks resident (arena reuses freed pools)
                w2c = []
                for fc in range(FT):
                    t_ = w2_pool.tile([128, D], BF16, tag=f"w2c{fc}", name=f"w2c{fc}")
                    nc.sync.dma_start(out=t_[:], in_=w2t_d[:, fc * D:(fc + 1) * D])
                    w2c.append(t_)

                # ---------------- back half ----------------
                wq = const.tile([M, T], BF16, tag="wqT")
                with tc.tile_pool(name="psW", bufs=1, space="PSUM") as psW, \
                     tc.tile_pool(name="psC", bufs=6, space="PSUM") as psC:
                    # k-sum GEMM: wq[m, t] = sum_j recv_j^T @ onehot_j
                    psw = psW.tile([M, T], F32, tag="psw")
                    for j in range(RJ):
                        blk = j // RPS      # (round, src) gather tile
                        jj = j % RPS
                        nc.tensor.matmul(psw[:], lhsT=recv[blk][:, jj * M:(jj + 1) * M],
                                         rhs=o2[j][:], start=(j == 0), stop=(j == RJ - 1))
                    nc.vector.tensor_copy(out=wq[:], in_=psw[:])
                    # pass2: psc = adj_chunk @ wq + I @ stage ; gelu PSUM -> stage
                    for ft in range(FT):
                        psc = psC.tile([128, T], F32, tag="psc")
                        nc.tensor.matmul(psc[:], lhsT=adj_sb[:, ft * 128:(ft + 1) * 128],
                                         rhs=wq[:], start=True, stop=False)
                        nc.tensor.matmul(psc[:], lhsT=ident[:],
                                         rhs=stage_tiles[ft][:],
                                         start=False, stop=True)
                        nc.scalar.activation(out=stage_tiles[ft][:], in_=psc[:],
                                             func=AF.Gelu)

                # W2 (fc-outer): psO[(q,dd)] accumulate across all fc
                with tc.tile_pool(name="psO", bufs=1, space="PSUM") as psO_pool:
                    psO = {}
                    for q in range(TT):
                        for dd in range(2):
                            psO[(q, dd)] = psO_pool.tile(
                                [128, 512], F32, tag=f"o{q}_{dd}", name=f"ops{q}_{dd}")
                    for fc in range(FT):
                        for q in range(TT):
                            for dd in range(2):
                                nc.tensor.matmul(
                                    psO[(q, dd)][:],
                                    lhsT=stage_tiles[fc][:, q * 128:(q + 1) * 128],
                                    rhs=w2c[fc][:, dd * 512:(dd + 1) * 512],
                                    start=(fc == 0), stop=(fc == FT - 1))
                    for q in range(TT):
                        ob = out_pool.tile([128, D], BF16, tag="ob")
                        for dd in range(2):
                            nc.vector.tensor_copy(out=ob[:, dd * 512:(dd + 1) * 512],
                                                  in_=psO[(q, dd)][:])
                        nc.scalar.dma_start(out=out_d[q * 128:(q + 1) * 128, :], in_=ob[:])

    nc.compile()
    return nc


def _routing(idx):
    """Group (t, k) pairs by pool entry; per-core slot packing."""
    flat_e = idx.ravel()
    order = np.argsort(flat_e, kind="stable")  # pairs sorted by (expert, t, k)
    counts = np.bincount(flat_e, minlength=POOL)
    starts = np.zeros(POOL, dtype=np.int64)
    starts[1:] = np.cumsum(counts)[:-1]
    tok_sorted = (np.arange(NTOK * K, dtype=np.int64) // K)[order]

    slot_expert = np.zeros((NCORES, EPC), dtype=np.int64)
    for c in range(NCORES):
        cnt = counts[c * EPC:(c + 1) * EPC]
        slot_expert[c] = c * EPC + np.argsort(-cnt, kind="stable")
    slot_counts_pc = counts[slot_expert]                    # [NCORES, EPC]
    slot_sizes = ((slot_counts_pc.max(axis=0) + 15) // 16 * 16).astype(np.int64)
    slot_sizes = np.maximum(slot_sizes, 16)
    assert slot_sizes.max() <= 128, f"slot overflow {slot_sizes.max()}"
    slot_off = np.concatenate([[0], np.cumsum(slot_sizes)])
    TW = int(slot_off[-1])
    return order, counts, starts, tok_sorted, slot_expert, slot_sizes, slot_off, TW


def _prepare_inputs(x, selected_indices, pattern_weights, base_patterns, cm_w, cm_b,
                    adj_proj, w2_w):
    bf = ml_dtypes.bfloat16
    x2 = np.ascontiguousarray(x.reshape(NTOK, D), dtype=np.float32)
    idx = np.ascontiguousarray(selected_indices.reshape(NTOK, K)).astype(np.int32)
    pw = np.ascontiguousarray(pattern_weights.reshape(NTOK, K), dtype=np.float32)

    pw_m = pw - pw.max(axis=1, keepdims=True)
    e = np.exp(pw_m)
    w = (e / e.sum(axis=1, keepdims=True)).astype(np.float32)      # [NTOK, K]

    bp_eff = base_patterns.astype(np.float32) + cm_b.reshape(POOL, M).astype(np.float32) @ adj_proj.astype(np.float32)
    bp_bf = bp_eff.astype(bf)
    adj_bf = adj_proj.astype(bf)
    x2t = x2.T                                                     # [D, NTOK] f32

    (order, counts, starts, tok_sorted, slot_expert, slot_sizes, slot_off,
     TW) = _routing(idx)
    ho = slot_off[::HGRP]
    w_sorted = w.ravel()[order]

    bp_t = np.ascontiguousarray(
        bp_bf.reshape(PC, 128, DFF).transpose(1, 0, 2).reshape(128, PC * DFF))
    w2t = np.ascontiguousarray(
        w2_w.T.astype(bf).reshape(FT, 128, D).transpose(1, 0, 2).reshape(128, FT * D))

    cm3 = cm_w.reshape(POOL, M, D)
    zeros_a2a = np.zeros((RTOT + 128, M), dtype=bf)

    # routing tables: scatter rows (per src core) + receiver token map
    tokx_all = np.full((NCORES, NR, NCORES, PADR), T + 88, dtype=np.float32)
    sidx_all = np.tile((RTOT + np.arange(128, dtype=np.int32))[None, :, None],
                       (NCORES, 1, EPC))
    for c in range(NCORES):
        for r in range(NR):
            rcount = np.zeros(NCORES, dtype=np.int64)
            for sl in range(r * SPR, (r + 1) * SPR):
                e_ = int(slot_expert[c, sl])
                cnt = int(counts[e_])
                toks = tok_sorted[starts[e_]:starts[e_] + cnt]
                d = toks // T
                for i in range(cnt):
                    di = int(d[i])
                    rr = rcount[di]; rcount[di] += 1
                    sidx_all[c, i, sl] = di * PADR + rr
                    tokx_all[di, r, c, rr] = float(toks[i] - di * T)
            assert rcount.max() <= PADR, f"A2A block overflow {rcount.max()}"

    in_maps = []
    for c in range(NCORES):
        xgt = np.zeros((128, DC * TW), dtype=bf)
        cmt = np.empty((128, EPC * DC * M), dtype=bf)
        for hi in range(2 * NG):
            hw = int(ho[hi + 1] - ho[hi])
            blk = np.zeros((D, hw), dtype=np.float32)
            for si in range(HGRP):
                sl = hi * HGRP + si
                e_ = int(slot_expert[c, sl])
                seg = slice(starts[e_], starts[e_] + counts[e_])
                toks = tok_sorted[seg]
                lo = int(slot_off[sl] - ho[hi])
                blk[:, lo:lo + len(toks)] = x2t[:, toks] * w_sorted[seg][None, :]
            xgt[:, DC * ho[hi]:DC * ho[hi + 1]] = (
                blk.reshape(DC, 128, hw).transpose(1, 0, 2).reshape(128, DC * hw)
            ).astype(bf)
        for sl in range(EPC):
            e_ = int(slot_expert[c, sl])
            cmt[:, sl * DC * M:(sl + 1) * DC * M] = (
                cm3[e_].T.reshape(DC, 128, M).transpose(1, 0, 2).reshape(128, DC * M)
            ).astype(bf)

        at = np.zeros((POOL, T), dtype=np.float32)
        tl = np.arange(c * T, (c + 1) * T)
        for k in range(K):
            np.add.at(at, (idx[tl, k], np.arange(T)), w[tl, k])
        atT = np.ascontiguousarray(
            at.astype(bf).reshape(PC, 128, T).transpose(1, 0, 2).reshape(128, PC * T))

        # gather offsets: partition p pulls rows srcc*RTOT + c*PADR + p*RPS ..
        gof = np.empty((128, NR * NCORES), dtype=np.int32)
        for r in range(NR):
            for srcc in range(NCORES):
                gof[:, r * NCORES + srcc] = (srcc * RTOT + c * PADR
                                             + np.arange(128) * RPS)
        # token map in ksum-chunk layout: chunk j = (r, srcc, jj);
        # value = token of recv row (r, srcc, p*RPS + jj) for dest c
        tokx = np.empty((128, RJ), dtype=np.float32)
        for r in range(NR):
            for srcc in range(NCORES):
                blkv = tokx_all[c, r, srcc].reshape(128, RPS)   # [p, jj]
                for jj in range(RPS):
                    tokx[:, r * RJC + srcc * RPS + jj] = blkv[:, jj]

        in_maps.append({
            "xgt": xgt,
            "cmt": np.ascontiguousarray(cmt),
            "bp": bp_t,
            "atT": atT,
            "adjp": adj_bf,
            "w2t": w2t,
            "sidx": np.ascontiguousarray(sidx_all[c]),
            "gof": gof,
            "tokx": tokx,
            "a2ain": zeros_a2a,
        })
    return in_maps, (slot_sizes, None)


def _run(inputs, trace=False):
    in_maps, slot_info = _prepare_inputs(
        inputs["x"], inputs["selected_indices"], inputs["pattern_weights"],
        inputs["base_patterns"], inputs["cm_w"], inputs["cm_b"],
        inputs["adj_proj"], inputs["w2_w"])
    nc = _build_program(slot_info)
    res = run_bass_kernel_spmd(nc, in_maps, core_ids=list(range(NCORES)), trace=trace)
    out = np.concatenate([res.results[c]["out"].astype(np.float32)
                          for c in range(NCORES)], axis=0)
    out = out + np.asarray(inputs["w2_b"], dtype=np.float32)[None, :]
    return out.reshape(B, S, D).astype(np.float32), res


def kernel(**inputs) -> np.ndarray:
    out, _ = _run(inputs, trace=False)
    return out
